# revision 15
# baseline (speedup 1.0000x reference)
"""Trainium2 Bass kernel for MessagePassingLayerEC (gnn_message_passing).

Math (reference):
    src_proj  = node_values @ W_src + b_src            # [V, D]
    dest_proj = node_values @ W_dest + b_dest          # [V, D]
    msgs = relu(src_proj[edge_src] + dest_proj[edge_dest] + edge_emb[edge_cls])
    out  = segment_sum(msgs, edge_dest, V)             # [V, D]

Strategy (8 cores, edge-parallel, dest-contiguous ownership => no all-reduce):
  - Host sorts edges by dest, splits at segment boundaries into 8 chunks; each
    core owns a disjoint contiguous range of dest rows.  Segments are packed
    into groups of <= 12*128 edges and <= 127 segments; a group's segment sums
    accumulate in one PSUM tile (via 0/1 segment-matrix matmuls) and are
    scattered to the output slab with one indirect DMA (unique rows).
  - Device phase 1 (bf16): each core computes src_proj for all V and
    dest_proj for its own dest rows.  Biases are folded into the edge-class
    embedding table on the host.  src_proj is stored at row offset +1 with
    row 0 zeroed and tail rows zero, so that out-of-half indices of the
    split gather can be clipped to a zero row.
  - Device phase 2: per super-batch of 48 edge-tiles, three dma_gathers
    (int16 ANT ucode): src rows from the low half-table, src rows from the
    high half-table (out-of-half indices clipped to zero rows), dest rows
    from the slab.  Merged with bf16 DVE adds + one-hot matmul embedding in
    PSUM, ReLU on ACT -> messages (bf16); per-tile segment matmuls -> PSUM;
    per-group indirect-DMA scatter.
"""

import sys

if "/opt/trn_rl_repo" not in sys.path:
    sys.path.insert(0, "/opt/trn_rl_repo")

import numpy as np
import ml_dtypes

BF16 = ml_dtypes.bfloat16

P = 128          # partitions / tile edge count
TPG = 12         # tiles per group
GE = TPG * P     # edges per group (1536)
MAXSEG = 127     # segments per group (slot 127 reserved for scratch)
SB_G = 4         # groups per gather super-batch
SBT = SB_G * TPG # tiles per super-batch (48)
NC_CORES = 8
SPLIT = 25000    # src half-table split point

V_GLOBAL = 50000
E_GLOBAL = 640000
DIM = 128
NCLS = 32


def _round_up(x, m):
    return (x + m - 1) // m * m


def _wrap_idx16(flat):
    """dma_gather index layout: idx j -> [j%16, j//16], replicated 8x down
    partitions; packed into int32 pairs for PJRT friendliness."""
    n = flat.shape[0]
    assert n % 32 == 0
    w = np.zeros((P, n // 16), dtype=np.int16)
    blk = flat.reshape(n // 16, 16).T
    for g in range(8):
        w[g * 16:(g + 1) * 16, :] = blk
    return np.ascontiguousarray(w).view(np.int32)


# ---------------------------------------------------------------------------
# Host-side packing
# ---------------------------------------------------------------------------

def _host_pack(node_values, edge_src, edge_dest, edge_cls,
               W_src, b_src, W_dest, b_dest, edge_emb, n_cores=NC_CORES):
    V, D = node_values.shape
    E = edge_src.shape[0]

    order = np.argsort(edge_dest, kind="stable")
    ds_ = edge_dest[order].astype(np.int64)
    ss_ = edge_src[order].astype(np.int32)
    cs_ = edge_cls[order].astype(np.int32)

    first = np.empty(E, dtype=bool)
    first[0] = True
    first[1:] = ds_[1:] != ds_[:-1]
    seg_starts = np.flatnonzero(first)
    nseg = len(seg_starts)
    seg_ends = np.append(seg_starts[1:], E)
    seg_lens = (seg_ends - seg_starts).astype(np.int64)
    seg_dest = ds_[seg_starts]
    assert seg_lens.max() <= GE, "segment larger than a group"

    # split segments into n_cores chunks with ~equal edge counts
    seg_cut = [0]
    for k in range(1, n_cores):
        tgt = k * E // n_cores
        i = np.searchsorted(seg_starts, tgt)
        i = min(max(i, 1), nseg - 1)
        if i > 1 and abs(int(seg_starts[i - 1]) - tgt) < abs(int(seg_starts[i]) - tgt):
            i -= 1
        seg_cut.append(i)
    seg_cut.append(nseg)

    # greedy sequential group packing per core
    core_groups = []
    for k in range(n_cores):
        lo, hi = seg_cut[k], seg_cut[k + 1]
        groups = []
        g_lo = lo
        cur_e = 0
        for g in range(lo, hi):
            L = int(seg_lens[g])
            if g > g_lo and (cur_e + L > GE or (g - g_lo) >= MAXSEG):
                groups.append((g_lo, g))
                g_lo = g
                cur_e = 0
            cur_e += L
        groups.append((g_lo, hi))
        core_groups.append(groups)

    NG = _round_up(max(len(g) for g in core_groups), SB_G)
    NT = NG * TPG
    n_rows = [seg_cut[k + 1] - seg_cut[k] for k in range(n_cores)]
    SLABP = _round_up(max(n_rows), 512)
    SCRATCH = SLABP
    OUT_ROWS = SLABP + P
    VP = _round_up(V + 1, 512)      # nodesT columns; proj rows [1, VP+1)
    split = min(SPLIT, max(1, V // 2))
    BCLIP = V + 1 - split           # zero row of the high half-table (idxB)
    assert split <= 32767 and BCLIP <= 32767 and VP + 1 - split <= 32768

    nodesT = np.zeros((D, VP), dtype=BF16)
    nodesT[:, :V] = np.ascontiguousarray(node_values.T).astype(BF16)

    emb_eff = (edge_emb + b_src[None, :] + b_dest[None, :]).astype(np.float32)
    emb_hi = emb_eff.astype(BF16)
    emb_lo = (emb_eff - emb_hi.astype(np.float32)).astype(BF16)

    iota_t = np.tile(np.arange(P, dtype=np.float32), (P, 1)).astype(BF16)

    in_maps = []
    asm = []
    for k in range(n_cores):
        lo, hi = seg_cut[k], seg_cut[k + 1]
        groups = core_groups[k]

        src_flat = np.full(NT * P, -1, dtype=np.int64)   # -1 marks padding
        dstl_flat = np.zeros(NT * P, dtype=np.int64)
        segid_flat = np.full(NT * P, MAXSEG, dtype=np.float32)
        cls_flat = np.zeros(NT * P, dtype=np.int64)
        scat = np.full((P, NG), SCRATCH, dtype=np.int32)

        for gi, (glo, ghi) in enumerate(groups):
            base = gi * GE
            pos = 0
            for slot, g in enumerate(range(glo, ghi)):
                L = int(seg_lens[g])
                sl = slice(int(seg_starts[g]), int(seg_ends[g]))
                fl = slice(base + pos, base + pos + L)
                src_flat[fl] = ss_[sl]
                cls_flat[fl] = cs_[sl]
                dstl_flat[fl] = g - lo
                segid_flat[fl] = slot
                scat[slot, gi] = g - lo
                pos += L
            assert pos <= GE

        pad = src_flat < 0
        low = (~pad) & (src_flat < split)
        high = (~pad) & (src_flat >= split)
        idxA = np.zeros(NT * P, dtype=np.int16)          # clip -> zero row 0
        idxA[low] = (src_flat[low] + 1).astype(np.int16)
        idxB = np.full(NT * P, BCLIP, dtype=np.int16)    # clip -> zero row
        idxB[high] = (src_flat[high] + 1 - split).astype(np.int16)
        idxC = np.zeros(NT * P, dtype=np.int16)
        idxC[~pad] = dstl_flat[~pad].astype(np.int16)

        onehotT = (cls_flat[None, :] == np.arange(NCLS, dtype=np.int64)[:, None])
        onehotT = np.ascontiguousarray(onehotT).astype(BF16)
        segid_pt = np.ascontiguousarray(
            segid_flat.reshape(NT, P).T)                 # [128, NT] f32

        nodesT_slab = np.zeros((D, SLABP), dtype=BF16)
        rows = seg_dest[lo:hi].astype(np.int64)
        nodesT_slab[:, : hi - lo] = nodesT[:, :V][:, rows]

        in_maps.append({
            "nodesT": nodesT,
            "nodesT_slab": nodesT_slab,
            "W_src": np.ascontiguousarray(W_src).astype(BF16),
            "W_dest": np.ascontiguousarray(W_dest).astype(BF16),
            "emb_hi": emb_hi,
            "emb_lo": emb_lo,
            "iota_t": iota_t,
            "idxA": _wrap_idx16(idxA),
            "idxB": _wrap_idx16(idxB),
            "idxC": _wrap_idx16(idxC),
            "segid": segid_pt,
            "onehotT": onehotT,
            "scat_idx": np.ascontiguousarray(scat),
        })
        asm.append(rows)

    params = dict(NG=int(NG), NT=int(NT), SLABP=int(SLABP),
                  OUT_ROWS=int(OUT_ROWS), VP=int(VP), D=int(D),
                  SPLIT=int(split))
    return in_maps, asm, params


# ---------------------------------------------------------------------------
# Bass kernel
# ---------------------------------------------------------------------------

def build_kernel(params, debug=False):
    import concourse.bass as bass
    import concourse.mybir as mybir
    import concourse.tile as tile
    from concourse import bacc

    NG = params["NG"]
    NT = params["NT"]
    SLABP = params["SLABP"]
    OUT_ROWS = params["OUT_ROWS"]
    VP = params["VP"]
    D = params["D"]
    NSB = NG // SB_G
    split = params["SPLIT"]
    TBL = VP + 1                      # src table rows (row 0 zero, +1 offset)
    NW = NT * P // 32                 # packed idx columns

    f32 = mybir.dt.float32
    bf16 = mybir.dt.bfloat16
    i32 = mybir.dt.int32
    i16 = mybir.dt.int16

    nc = bacc.Bacc("TRN2", target_bir_lowering=False)

    nodesT = nc.dram_tensor("nodesT", [D, VP], bf16, kind="ExternalInput")
    nodesT_slab = nc.dram_tensor("nodesT_slab", [D, SLABP], bf16, kind="ExternalInput")
    W_src = nc.dram_tensor("W_src", [D, D], bf16, kind="ExternalInput")
    W_dest = nc.dram_tensor("W_dest", [D, D], bf16, kind="ExternalInput")
    emb_hi = nc.dram_tensor("emb_hi", [NCLS, D], bf16, kind="ExternalInput")
    emb_lo = nc.dram_tensor("emb_lo", [NCLS, D], bf16, kind="ExternalInput")
    iota_t = nc.dram_tensor("iota_t", [P, P], bf16, kind="ExternalInput")
    idxA = nc.dram_tensor("idxA", [P, NW], i32, kind="ExternalInput")
    idxB = nc.dram_tensor("idxB", [P, NW], i32, kind="ExternalInput")
    idxC = nc.dram_tensor("idxC", [P, NW], i32, kind="ExternalInput")
    segid = nc.dram_tensor("segid", [P, NT], f32, kind="ExternalInput")
    onehotT = nc.dram_tensor("onehotT", [NCLS, NT * P], bf16, kind="ExternalInput")
    scat_idx = nc.dram_tensor("scat_idx", [P, NG], i32, kind="ExternalInput")

    kind_dbg = "ExternalOutput" if debug else "Internal"
    src_tbl = nc.dram_tensor("src_tbl", [TBL, D], bf16, kind=kind_dbg)
    dest_proj = nc.dram_tensor("dest_proj", [SLABP, D], bf16, kind=kind_dbg)
    out = nc.dram_tensor("out", [OUT_ROWS, D], f32, kind="ExternalOutput")
    dbg_gsum = (nc.dram_tensor("dbg_gsum", [P, SBT * D], f32, kind="ExternalOutput")
                if debug else None)

    with tile.TileContext(nc) as tc, tc.tile_pool(name="const", bufs=1) as cpool:
        w_src_sb = cpool.tile([D, D], bf16, tag="wsrc")
        nc.sync.dma_start(w_src_sb[:], W_src[:, :])
        w_dest_sb = cpool.tile([D, D], bf16, tag="wdest")
        nc.sync.dma_start(w_dest_sb[:], W_dest[:, :])
        emb_hi_sb = cpool.tile([NCLS, D], bf16, tag="embhi")
        nc.sync.dma_start(emb_hi_sb[:], emb_hi[:, :])
        emb_lo_sb = cpool.tile([NCLS, D], bf16, tag="emblo")
        nc.sync.dma_start(emb_lo_sb[:], emb_lo[:, :])
        iota_sb = cpool.tile([P, P], bf16, tag="iota")
        nc.sync.dma_start(iota_sb[:], iota_t[:, :])

        # ---------------- phase 1: projections (bf16) ----------------
        with (
            tc.tile_pool(name="p1", bufs=3) as p1pool,
            tc.tile_pool(name="p1ps", bufs=2, space="PSUM") as p1ps,
        ):
            zrow = p1pool.tile([1, D], bf16, tag="zrow")
            nc.vector.memset(zrow[:], 0.0)
            nc.sync.dma_start(src_tbl[0:1, :], zrow[:])

            def proj_pass(n_rows, src_dram, w_sb, dview):
                nchunk = n_rows // 512
                for ch in range(nchunk):
                    nt_sb = p1pool.tile([D, 512], bf16, tag="p1in")
                    nc.sync.dma_start(nt_sb[:], src_dram[:, ch * 512:(ch + 1) * 512])
                    ps = p1ps.tile([P, 512], f32, tag="p1ps")
                    for j in range(4):
                        nc.tensor.matmul(
                            ps[:, j * P:(j + 1) * P],
                            lhsT=nt_sb[:, j * P:(j + 1) * P],
                            rhs=w_sb[:],
                            start=True, stop=True,
                        )
                    ob = p1pool.tile([P, 512], bf16, tag="p1out")
                    nc.vector.tensor_copy(ob[:], ps[:])
                    nc.sync.dma_start(
                        dview[:, ch * 4:(ch + 1) * 4, :],
                        ob[:].rearrange("p (c d) -> p c d", d=D),
                    )

            proj_pass(VP, nodesT, w_src_sb,
                      src_tbl[1:TBL, :].rearrange("(c p) d -> p c d", p=P))
            proj_pass(SLABP, nodesT_slab, w_dest_sb,
                      dest_proj[:, :].rearrange("(c p) d -> p c d", p=P))

        # ---------------- phase 2: edges ----------------
        with (
            tc.tile_pool(name="meta", bufs=3) as mpool,
            tc.tile_pool(name="gath", bufs=2) as gpool,
            tc.tile_pool(name="work", bufs=4) as wpool,
            tc.tile_pool(name="msgs", bufs=2) as mspool,
            tc.tile_pool(name="segout", bufs=3) as spool,
            tc.tile_pool(name="psmsg", bufs=3, space="PSUM") as psmsg,
            tc.tile_pool(name="psseg", bufs=2, space="PSUM") as psseg,
        ):
            NWSB = SBT * P // 32
            for sb in range(NSB):
                t0 = sb * SBT
                ia = mpool.tile([P, NWSB], i32, tag="ia")
                nc.sync.dma_start(ia[:], idxA[:, sb * NWSB:(sb + 1) * NWSB])
                ib = mpool.tile([P, NWSB], i32, tag="ib")
                nc.sync.dma_start(ib[:], idxB[:, sb * NWSB:(sb + 1) * NWSB])
                ic = mpool.tile([P, NWSB], i32, tag="ic")
                nc.sync.dma_start(ic[:], idxC[:, sb * NWSB:(sb + 1) * NWSB])
                sgid = mpool.tile([P, SBT], f32, tag="sgid")
                nc.sync.dma_start(sgid[:], segid[:, t0:t0 + SBT])
                oht = mpool.tile([NCLS, SBT * P], bf16, tag="oht")
                nc.sync.dma_start(oht[:], onehotT[:, t0 * P:(t0 + SBT) * P])
                scat = mpool.tile([P, SB_G], i32, tag="scat")
                nc.sync.dma_start(scat[:], scat_idx[:, sb * SB_G:(sb + 1) * SB_G])

                NIDX = SBT * P
                ga = gpool.tile([P, SBT, D], bf16, tag="ga")
                nc.gpsimd.dma_gather(
                    ga[:], src_tbl[:, :], ia[:].bitcast(i16),
                    NIDX, NIDX, D, single_packet=False)
                gb = gpool.tile([P, SBT, D], bf16, tag="gb")
                nc.gpsimd.dma_gather(
                    gb[:], src_tbl[split:TBL, :], ib[:].bitcast(i16),
                    NIDX, NIDX, D, single_packet=False)
                gc = gpool.tile([P, SBT, D], bf16, tag="gc")
                nc.gpsimd.dma_gather(
                    gc[:], dest_proj[:, :], ic[:].bitcast(i16),
                    NIDX, NIDX, D, single_packet=False)

                for g in range(SB_G):
                    ps_seg = psseg.tile([P, D], f32, tag="psseg")
                    for c in range(TPG // 4):
                        tloc = g * TPG + c * 4
                        ps_m = psmsg.tile([P, 512], f32, tag="psmsg")
                        for j in range(4):
                            colsl = slice((tloc + j) * P, (tloc + j + 1) * P)
                            nc.tensor.matmul(
                                ps_m[:, j * P:(j + 1) * P],
                                lhsT=oht[:, colsl], rhs=emb_hi_sb[:],
                                start=True, stop=False,
                            )
                            nc.tensor.matmul(
                                ps_m[:, j * P:(j + 1) * P],
                                lhsT=oht[:, colsl], rhs=emb_lo_sb[:],
                                start=False, stop=True,
                            )
                        sl3 = lambda t_: (slice(None), slice(t_, t_ + 4),
                                          slice(None))
                        t1 = wpool.tile([P, 512], bf16, tag="t1")
                        nc.vector.tensor_tensor(
                            out=t1[:],
                            in0=ga[sl3(tloc)].rearrange("p c d -> p (c d)"),
                            in1=gb[sl3(tloc)].rearrange("p c d -> p (c d)"),
                            op=mybir.AluOpType.add)
                        t2 = wpool.tile([P, 512], bf16, tag="t2")
                        nc.vector.tensor_tensor(
                            out=t2[:], in0=t1[:],
                            in1=gc[sl3(tloc)].rearrange("p c d -> p (c d)"),
                            op=mybir.AluOpType.add)
                        t3 = wpool.tile([P, 512], f32, tag="t3")
                        nc.vector.tensor_tensor(
                            out=t3[:], in0=t2[:], in1=ps_m[:],
                            op=mybir.AluOpType.add)
                        if debug and sb == 0:
                            nc.sync.dma_start(
                                dbg_gsum[:, tloc * D:(tloc + 4) * D], t3[:])
                        msgs = mspool.tile([P, 512], bf16, tag="msgs")
                        nc.scalar.activation(
                            msgs[:], t3[:], mybir.ActivationFunctionType.Relu)
                        for j in range(4):
                            t = tloc + j
                            gt = wpool.tile([P, P], bf16, tag="gt")
                            nc.any.tensor_scalar(
                                out=gt[:], in0=iota_sb[:],
                                scalar1=sgid[:, t:t + 1], scalar2=None,
                                op0=mybir.AluOpType.is_equal)
                            nc.tensor.matmul(
                                ps_seg[:],
                                lhsT=gt[:], rhs=msgs[:, j * P:(j + 1) * P],
                                start=(c == 0 and j == 0),
                                stop=(c == 2 and j == 3))
                    seg_sb = spool.tile([P, D], f32, tag="segsb")
                    nc.vector.tensor_copy(seg_sb[:], ps_seg[:])
                    nc.gpsimd.indirect_dma_start(
                        out=out[:, :],
                        out_offset=bass.IndirectOffsetOnAxis(
                            ap=scat[:, g:g + 1], axis=0),
                        in_=seg_sb[:],
                        in_offset=None)

    nc.compile()
    return nc


# ---------------------------------------------------------------------------
# Entry point
# ---------------------------------------------------------------------------

def kernel(**inputs):
    node_values = np.asarray(inputs["node_values"], dtype=np.float32)
    edge_src = np.asarray(inputs["edge_src"], dtype=np.int32)
    edge_dest = np.asarray(inputs["edge_dest"], dtype=np.int32)
    edge_cls = np.asarray(inputs["edge_cls"], dtype=np.int32)
    W_src = np.asarray(inputs["W_src"], dtype=np.float32)
    b_src = np.asarray(inputs["b_src"], dtype=np.float32)
    W_dest = np.asarray(inputs["W_dest"], dtype=np.float32)
    b_dest = np.asarray(inputs["b_dest"], dtype=np.float32)
    edge_emb = np.asarray(inputs["edge_emb"], dtype=np.float32)

    V = node_values.shape[0]

    in_maps, asm, params = _host_pack(
        node_values, edge_src, edge_dest, edge_cls,
        W_src, b_src, W_dest, b_dest, edge_emb)

    nc = build_kernel(params)

    from concourse.bass_utils import run_bass_kernel_spmd
    res = run_bass_kernel_spmd(nc, in_maps, core_ids=list(range(NC_CORES)))

    out = np.zeros((V, DIM), dtype=np.float32)
    for k in range(NC_CORES):
        rows = asm[k]
        out[rows] = np.asarray(res.results[k]["out"])[: len(rows)]
    return out


if __name__ == "__main__":
    rng = np.random.default_rng(0)
    V, E = V_GLOBAL, E_GLOBAL
    ins = {
        "node_values": rng.normal(size=(V, DIM)).astype(np.float32),
        "edge_src": rng.integers(0, V, size=E).astype(np.int32),
        "edge_dest": rng.integers(0, V, size=E).astype(np.int32),
        "edge_cls": rng.integers(0, NCLS, size=E).astype(np.int32),
        "W_src": (rng.normal(size=(DIM, DIM)) / np.sqrt(DIM)).astype(np.float32),
        "b_src": np.zeros(DIM, dtype=np.float32),
        "W_dest": (rng.normal(size=(DIM, DIM)) / np.sqrt(DIM)).astype(np.float32),
        "b_dest": np.zeros(DIM, dtype=np.float32),
        "edge_emb": rng.normal(size=(NCLS, DIM)).astype(np.float32),
    }
    out = kernel(**ins)
    print("out", out.shape, out.dtype, float(np.abs(out).sum()))


# revision 21
# speedup vs baseline: 35.2756x; 35.2756x over previous
"""Trainium2 Bass kernel for MessagePassingLayerEC (gnn_message_passing).

Math (reference):
    src_proj  = node_values @ W_src + b_src            # [V, D]
    dest_proj = node_values @ W_dest + b_dest          # [V, D]
    msgs = relu(src_proj[edge_src] + dest_proj[edge_dest] + edge_emb[edge_cls])
    out  = segment_sum(msgs, edge_dest, V)             # [V, D]

Strategy (8 cores, edge-parallel, dest-contiguous ownership => no all-reduce):
  - Host sorts edges by dest, splits at segment boundaries into 8 chunks; each
    core owns a disjoint contiguous range of dest rows.  Segments are packed
    into groups of <= 12*128 edges and <= 127 segments; a group's segment sums
    accumulate in one PSUM tile (via 0/1 segment-matrix matmuls) and are
    scattered to the output slab with one indirect DMA (unique rows).
  - Device phase 1 (bf16): each core computes src_proj for all V and
    dest_proj for its own dest rows.  Biases are folded into the edge-class
    embedding table on the host.  src_proj is stored at row offset +1 with
    row 0 zeroed and tail rows zero, so that out-of-half indices of the
    split gather can be clipped to a zero row.
  - Device phase 2: per super-batch of 48 edge-tiles, three dma_gathers
    (int16 ANT ucode): src rows from the low half-table, src rows from the
    high half-table (out-of-half indices clipped to zero rows), dest rows
    from the slab.  Merged with bf16 DVE adds + one-hot matmul embedding in
    PSUM, ReLU on ACT -> messages (bf16); per-tile segment matmuls -> PSUM;
    per-group indirect-DMA scatter.
"""

import sys

if "/opt/trn_rl_repo" not in sys.path:
    sys.path.insert(0, "/opt/trn_rl_repo")

import numpy as np
import ml_dtypes

BF16 = ml_dtypes.bfloat16

P = 128          # partitions / tile edge count
TPG = 12         # tiles per group
GE = TPG * P     # edges per group (1536)
MAXSEG = 127     # segments per group (slot 127 reserved for scratch)
SB_G = 4         # groups per gather super-batch
SBT = SB_G * TPG # tiles per super-batch (48)
NC_CORES = 8
SPLIT = 25000    # src half-table split point

V_GLOBAL = 50000
E_GLOBAL = 640000
DIM = 128
NCLS = 32


def _round_up(x, m):
    return (x + m - 1) // m * m


def _wrap_idx16(flat):
    """dma_gather index layout: idx j -> [j%16, j//16], replicated 8x down
    partitions; packed into int32 pairs for PJRT friendliness."""
    n = flat.shape[0]
    assert n % 32 == 0
    w = np.zeros((P, n // 16), dtype=np.int16)
    blk = flat.reshape(n // 16, 16).T
    for g in range(8):
        w[g * 16:(g + 1) * 16, :] = blk
    return np.ascontiguousarray(w).view(np.int32)


# ---------------------------------------------------------------------------
# Host-side packing
# ---------------------------------------------------------------------------

def _host_pack(node_values, edge_src, edge_dest, edge_cls,
               W_src, b_src, W_dest, b_dest, edge_emb, n_cores=NC_CORES):
    V, D = node_values.shape
    E = edge_src.shape[0]

    order = np.argsort(edge_dest, kind="stable")
    ds_ = edge_dest[order].astype(np.int64)
    ss_ = edge_src[order].astype(np.int32)
    cs_ = edge_cls[order].astype(np.int32)

    first = np.empty(E, dtype=bool)
    first[0] = True
    first[1:] = ds_[1:] != ds_[:-1]
    seg_starts = np.flatnonzero(first)
    nseg = len(seg_starts)
    seg_ends = np.append(seg_starts[1:], E)
    seg_lens = (seg_ends - seg_starts).astype(np.int64)
    seg_dest = ds_[seg_starts]
    assert seg_lens.max() <= GE, "segment larger than a group"

    # split segments into n_cores chunks with ~equal edge counts
    seg_cut = [0]
    for k in range(1, n_cores):
        tgt = k * E // n_cores
        i = np.searchsorted(seg_starts, tgt)
        i = min(max(i, 1), nseg - 1)
        if i > 1 and abs(int(seg_starts[i - 1]) - tgt) < abs(int(seg_starts[i]) - tgt):
            i -= 1
        seg_cut.append(i)
    seg_cut.append(nseg)

    # greedy sequential group packing per core
    core_groups = []
    for k in range(n_cores):
        lo, hi = seg_cut[k], seg_cut[k + 1]
        groups = []
        g_lo = lo
        cur_e = 0
        for g in range(lo, hi):
            L = int(seg_lens[g])
            if g > g_lo and (cur_e + L > GE or (g - g_lo) >= MAXSEG):
                groups.append((g_lo, g))
                g_lo = g
                cur_e = 0
            cur_e += L
        groups.append((g_lo, hi))
        core_groups.append(groups)

    NG = _round_up(max(len(g) for g in core_groups), SB_G)
    NT = NG * TPG
    n_rows = [seg_cut[k + 1] - seg_cut[k] for k in range(n_cores)]
    SLABP = _round_up(max(n_rows), 2048)
    SCRATCH = SLABP
    OUT_ROWS = SLABP + 2048
    VP = _round_up(V + 1, 2048)     # nodesT columns; proj rows [1, VP+1)
    split = min(SPLIT, max(1, V // 2))
    BCLIP = V + 1 - split           # zero row of the high half-table (idxB)
    assert split <= 32767 and BCLIP <= 32767 and VP + 1 - split <= 32768

    nodesT = np.zeros((D, VP), dtype=BF16)
    nodesT[:, :V] = np.ascontiguousarray(node_values.T).astype(BF16)

    def _perm_cols(tbl):
        # column (c*512 + j*128 + p) <- node (c*512 + 4p + j): makes each
        # phase-1 output partition hold 4 consecutive rows (1KB descriptors)
        n = tbl.shape[1]
        pos = np.arange(n)
        node = (pos // 512) * 512 + 4 * (pos % 128) + (pos // 128) % 4
        return np.ascontiguousarray(tbl[:, node])

    emb_eff = (edge_emb + b_src[None, :] + b_dest[None, :]).astype(np.float32)
    emb_hi = emb_eff.astype(BF16)

    iota_t = np.tile(np.arange(P, dtype=np.float32), (P, 1)).astype(BF16)

    nodesT_perm = _perm_cols(nodesT)
    in_maps = []
    asm = []
    for k in range(n_cores):
        lo, hi = seg_cut[k], seg_cut[k + 1]
        groups = core_groups[k]

        src_flat = np.full(NT * P, -1, dtype=np.int64)   # -1 marks padding
        dstl_flat = np.zeros(NT * P, dtype=np.int64)
        segid_flat = np.full(NT * P, MAXSEG, dtype=np.float32)
        cls_flat = np.zeros(NT * P, dtype=np.int64)
        scat = np.full((P, NG), SCRATCH, dtype=np.int32)

        for gi, (glo, ghi) in enumerate(groups):
            base = gi * GE
            pos = 0
            for slot, g in enumerate(range(glo, ghi)):
                L = int(seg_lens[g])
                sl = slice(int(seg_starts[g]), int(seg_ends[g]))
                fl = slice(base + pos, base + pos + L)
                src_flat[fl] = ss_[sl]
                cls_flat[fl] = cs_[sl]
                dstl_flat[fl] = g - lo
                segid_flat[fl] = slot
                scat[slot, gi] = g - lo
                pos += L
            assert pos <= GE

        pad = src_flat < 0
        low = (~pad) & (src_flat < split)
        high = (~pad) & (src_flat >= split)
        idxA = np.zeros(NT * P, dtype=np.int16)          # clip -> zero row 0
        idxA[low] = (src_flat[low] + 1).astype(np.int16)
        idxB = np.full(NT * P, BCLIP, dtype=np.int16)    # clip -> zero row
        idxB[high] = (src_flat[high] + 1 - split).astype(np.int16)
        idxC = np.zeros(NT * P, dtype=np.int16)
        idxC[~pad] = dstl_flat[~pad].astype(np.int16)

        onehotT = (cls_flat[None, :] == np.arange(NCLS, dtype=np.int64)[:, None])
        onehotT = np.ascontiguousarray(onehotT).astype(BF16)
        segid_pt = np.ascontiguousarray(
            segid_flat.reshape(NT, P).T)                 # [128, NT] f32

        nodesT_slab = np.zeros((D, SLABP), dtype=BF16)
        rows = seg_dest[lo:hi].astype(np.int64)
        nodesT_slab[:, : hi - lo] = nodesT[:, :V][:, rows]
        nodesT_slab = _perm_cols(nodesT_slab)

        # packed per-sb metadata: [idxA(192) idxB(192) idxC(192) segid(48)
        # scat16(16)] int32 columns per super-batch
        NSB_ = NG // SB_G
        wa, wb, wc = _wrap_idx16(idxA), _wrap_idx16(idxB), _wrap_idx16(idxC)
        SBW = SBT * P // 32
        meta = np.zeros((P, NSB_ * (3 * SBW + SBT + 16)), dtype=np.int32)
        MW = 3 * SBW + SBT + 16
        for sb in range(NSB_):
            c0 = sb * MW
            meta[:, c0:c0 + SBW] = wa[:, sb * SBW:(sb + 1) * SBW]
            meta[:, c0 + SBW:c0 + 2 * SBW] = wb[:, sb * SBW:(sb + 1) * SBW]
            meta[:, c0 + 2 * SBW:c0 + 3 * SBW] = wc[:, sb * SBW:(sb + 1) * SBW]
            meta[:, c0 + 3 * SBW:c0 + 3 * SBW + SBT] = \
                segid_pt[:, sb * SBT:(sb + 1) * SBT].view(np.int32)
            sflat = scat[:, sb * SB_G:(sb + 1) * SB_G].T.ravel().astype(np.int16)
            meta[:, c0 + 3 * SBW + SBT:c0 + MW] = _wrap_idx16(sflat)
        in_maps.append({
            "nodesT": nodesT_perm,
            "nodesT_slab": nodesT_slab,
            "W_src": np.ascontiguousarray(W_src).astype(BF16),
            "W_dest": np.ascontiguousarray(W_dest).astype(BF16),
            "emb_hi": emb_hi,
            "iota_t": iota_t,
            "meta": meta,
            "onehotT": onehotT,
        })
        asm.append(rows)

    params = dict(NG=int(NG), NT=int(NT), SLABP=int(SLABP),
                  OUT_ROWS=int(OUT_ROWS), VP=int(VP), D=int(D),
                  SPLIT=int(split))
    return in_maps, asm, params


# ---------------------------------------------------------------------------
# Bass kernel
# ---------------------------------------------------------------------------

def build_kernel(params, debug=False):
    import concourse.bass as bass
    import concourse.mybir as mybir
    import concourse.tile as tile
    from concourse import bacc

    NG = params["NG"]
    NT = params["NT"]
    SLABP = params["SLABP"]
    OUT_ROWS = params["OUT_ROWS"]
    VP = params["VP"]
    D = params["D"]
    NSB = NG // SB_G
    split = params["SPLIT"]
    TBL = VP + 1                      # src table rows (row 0 zero, +1 offset)
    NW = NT * P // 32                 # packed idx columns

    f32 = mybir.dt.float32
    bf16 = mybir.dt.bfloat16
    i32 = mybir.dt.int32
    i16 = mybir.dt.int16

    nc = bacc.Bacc("TRN2", target_bir_lowering=False)

    nodesT = nc.dram_tensor("nodesT", [D, VP], bf16, kind="ExternalInput")
    nodesT_slab = nc.dram_tensor("nodesT_slab", [D, SLABP], bf16, kind="ExternalInput")
    W_src = nc.dram_tensor("W_src", [D, D], bf16, kind="ExternalInput")
    W_dest = nc.dram_tensor("W_dest", [D, D], bf16, kind="ExternalInput")
    emb_hi = nc.dram_tensor("emb_hi", [NCLS, D], bf16, kind="ExternalInput")
    iota_t = nc.dram_tensor("iota_t", [P, P], bf16, kind="ExternalInput")
    SBW = SBT * P // 32
    MW = 3 * SBW + SBT + 16
    meta = nc.dram_tensor("meta", [P, NSB * MW], i32, kind="ExternalInput")
    onehotT = nc.dram_tensor("onehotT", [NCLS, NT * P], bf16, kind="ExternalInput")

    kind_dbg = "ExternalOutput" if debug else "Internal"
    src_tbl = nc.dram_tensor("src_tbl", [TBL, D], bf16, kind=kind_dbg)
    dest_proj = nc.dram_tensor("dest_proj", [SLABP, D], bf16, kind=kind_dbg)
    out = nc.dram_tensor("out", [OUT_ROWS, D], f32, kind="ExternalOutput")
    dbg_gsum = (nc.dram_tensor("dbg_gsum", [P, SBT * D], f32, kind="ExternalOutput")
                if debug else None)

    with tile.TileContext(nc) as tc, tc.tile_pool(name="const", bufs=1) as cpool:
        w_src_sb = cpool.tile([D, D], bf16, tag="wsrc")
        nc.sync.dma_start(w_src_sb[:], W_src[:, :])
        w_dest_sb = cpool.tile([D, D], bf16, tag="wdest")
        nc.sync.dma_start(w_dest_sb[:], W_dest[:, :])
        emb_hi_sb = cpool.tile([NCLS, D], bf16, tag="embhi")
        nc.sync.dma_start(emb_hi_sb[:], emb_hi[:, :])
        iota_sb = cpool.tile([P, P], bf16, tag="iota")
        nc.sync.dma_start(iota_sb[:], iota_t[:, :])

        # ---------------- phase 1: projections (bf16) ----------------
        with (
            tc.tile_pool(name="p1", bufs=3) as p1pool,
            tc.tile_pool(name="p1ps", bufs=2, space="PSUM") as p1ps,
        ):
            zrow = p1pool.tile([1, D], bf16, tag="zrow")
            nc.vector.memset(zrow[:], 0.0)
            nc.sync.dma_start(src_tbl[0:1, :], zrow[:])

            def proj_pass(n_rows, src_dram, w_sb, dview):
                nsup = n_rows // 2048
                for su in range(nsup):
                    nt_sb = p1pool.tile([D, 2048], bf16, tag="p1in")
                    nc.sync.dma_start(
                        nt_sb[:], src_dram[:, su * 2048:(su + 1) * 2048])
                    ob = p1pool.tile([P, 4, 512], bf16, tag="p1out")
                    for cc in range(4):
                        ps = p1ps.tile([P, 512], f32, tag="p1ps")
                        for j in range(4):
                            nc.tensor.matmul(
                                ps[:, j * P:(j + 1) * P],
                                lhsT=nt_sb[:, cc * 512 + j * P:
                                           cc * 512 + (j + 1) * P],
                                rhs=w_sb[:],
                                start=True, stop=True,
                            )
                        nc.scalar.activation(
                            ob[:, cc, :], ps[:],
                            mybir.ActivationFunctionType.Copy)
                    nc.sync.dma_start(
                        dview[:, su * 4:(su + 1) * 4, :], ob[:])

            proj_pass(VP, nodesT, w_src_sb,
                      src_tbl[1:TBL, :].rearrange("(c p r) d -> p c (r d)",
                                                  p=P, r=4))
            proj_pass(SLABP, nodesT_slab, w_dest_sb,
                      dest_proj[:, :].rearrange("(c p r) d -> p c (r d)",
                                                p=P, r=4))

        # zero the output slab (scatter-add target; degree-0 rows stay 0)
        with tc.tile_pool(name="zz", bufs=1) as zpool:
            zt = zpool.tile([P, 512], f32, tag="zt")
            nc.vector.memset(zt[:], 0.0)
            zview = out[:, :].rearrange("(c p r) d -> p c (r d)", p=P, r=4)
            for zc in range(OUT_ROWS // 512):
                nc.sync.dma_start(zview[:, zc:zc + 1, :],
                                  zt[:].rearrange("p (o f) -> p o f", o=1))

        # ---------------- phase 2: edges ----------------
        with (
            tc.tile_pool(name="meta", bufs=3) as mpool,
            tc.tile_pool(name="gath", bufs=2) as gpool,
            tc.tile_pool(name="work", bufs=4) as wpool,
            tc.tile_pool(name="msgs", bufs=2) as mspool,
            tc.tile_pool(name="segout", bufs=3) as spool,
            tc.tile_pool(name="psmsg", bufs=3, space="PSUM") as psmsg,
            tc.tile_pool(name="psseg", bufs=2, space="PSUM") as psseg,
        ):
            for sb in range(NSB):
                t0 = sb * SBT
                mt = mpool.tile([P, MW], i32, tag="mt")
                nc.sync.dma_start(mt[:], meta[:, sb * MW:(sb + 1) * MW])
                ia = mt[:, 0:SBW]
                ib = mt[:, SBW:2 * SBW]
                ic = mt[:, 2 * SBW:3 * SBW]
                sgid = mt[:, 3 * SBW:3 * SBW + SBT].bitcast(f32)
                sc16 = mt[:, 3 * SBW + SBT:MW]
                oht = mpool.tile([NCLS, SBT * P], bf16, tag="oht")
                nc.sync.dma_start(oht[:], onehotT[:, t0 * P:(t0 + SBT) * P])

                NIDX = SBT * P
                ga = gpool.tile([P, SBT, D], bf16, tag="ga")
                nc.gpsimd.dma_gather(
                    ga[:], src_tbl[:, :], ia.bitcast(i16),
                    NIDX, NIDX, D, single_packet=False)
                gb = gpool.tile([P, SBT, D], bf16, tag="gb")
                nc.gpsimd.dma_gather(
                    gb[:], src_tbl[split:TBL, :], ib.bitcast(i16),
                    NIDX, NIDX, D, single_packet=False)
                gc = gpool.tile([P, SBT, D], bf16, tag="gc")
                nc.gpsimd.dma_gather(
                    gc[:], dest_proj[:, :], ic.bitcast(i16),
                    NIDX, NIDX, D, single_packet=False)

                seg_sb = spool.tile([P, SB_G, D], f32, tag="segsb")
                for g in range(SB_G):
                    gsl = (slice(None), slice(g * TPG, (g + 1) * TPG),
                           slice(None))
                    t2 = wpool.tile([P, TPG * D], bf16, tag="t2")
                    nc.vector.tensor_tensor(
                        out=t2[:],
                        in0=ga[gsl].rearrange("p c d -> p (c d)"),
                        in1=gb[gsl].rearrange("p c d -> p (c d)"),
                        op=mybir.AluOpType.add)
                    nc.vector.tensor_tensor(
                        out=t2[:], in0=t2[:],
                        in1=gc[gsl].rearrange("p c d -> p (c d)"),
                        op=mybir.AluOpType.add)
                    ps_seg = psseg.tile([P, D], f32, tag="psseg")
                    for c in range(TPG // 4):
                        tloc = g * TPG + c * 4
                        ps_m = psmsg.tile([P, 512], f32, tag="psmsg")
                        for j in range(4):
                            colsl = slice((tloc + j) * P, (tloc + j + 1) * P)
                            nc.tensor.matmul(
                                ps_m[:, j * P:(j + 1) * P],
                                lhsT=oht[:, colsl], rhs=emb_hi_sb[:],
                                start=True, stop=True,
                            )
                        t3 = wpool.tile([P, 512], f32, tag="t3")
                        nc.vector.tensor_tensor(
                            out=t3[:], in0=t2[:, c * 512:(c + 1) * 512],
                            in1=ps_m[:],
                            op=mybir.AluOpType.add)
                        if debug and sb == 0:
                            nc.sync.dma_start(
                                dbg_gsum[:, tloc * D:(tloc + 4) * D], t3[:])
                        msgs = mspool.tile([P, 512], bf16, tag="msgs")
                        nc.scalar.activation(
                            msgs[:], t3[:], mybir.ActivationFunctionType.Relu)
                        for j in range(4):
                            t = tloc + j
                            gt = wpool.tile([P, P], bf16, tag="gt")
                            nc.any.tensor_scalar(
                                out=gt[:], in0=iota_sb[:],
                                scalar1=sgid[:, t:t + 1], scalar2=None,
                                op0=mybir.AluOpType.is_equal)
                            nc.tensor.matmul(
                                ps_seg[:],
                                lhsT=gt[:], rhs=msgs[:, j * P:(j + 1) * P],
                                start=(c == 0 and j == 0),
                                stop=(c == 2 and j == 3))
                    nc.any.tensor_copy(seg_sb[:, g, :], ps_seg[:])
                nc.gpsimd.dma_scatter_add(
                    out[:, :], seg_sb[:], sc16.bitcast(i16),
                    SB_G * P, SB_G * P, D, single_packet=False)

    nc.compile()
    return nc


# ---------------------------------------------------------------------------
# Entry point
# ---------------------------------------------------------------------------

def kernel(**inputs):
    node_values = np.asarray(inputs["node_values"], dtype=np.float32)
    edge_src = np.asarray(inputs["edge_src"], dtype=np.int32)
    edge_dest = np.asarray(inputs["edge_dest"], dtype=np.int32)
    edge_cls = np.asarray(inputs["edge_cls"], dtype=np.int32)
    W_src = np.asarray(inputs["W_src"], dtype=np.float32)
    b_src = np.asarray(inputs["b_src"], dtype=np.float32)
    W_dest = np.asarray(inputs["W_dest"], dtype=np.float32)
    b_dest = np.asarray(inputs["b_dest"], dtype=np.float32)
    edge_emb = np.asarray(inputs["edge_emb"], dtype=np.float32)

    V = node_values.shape[0]

    in_maps, asm, params = _host_pack(
        node_values, edge_src, edge_dest, edge_cls,
        W_src, b_src, W_dest, b_dest, edge_emb)

    nc = build_kernel(params)

    from concourse.bass_utils import run_bass_kernel_spmd
    res = run_bass_kernel_spmd(nc, in_maps, core_ids=list(range(NC_CORES)))

    out = np.zeros((V, DIM), dtype=np.float32)
    for k in range(NC_CORES):
        rows = asm[k]
        out[rows] = np.asarray(res.results[k]["out"])[: len(rows)]
    return out


if __name__ == "__main__":
    rng = np.random.default_rng(0)
    V, E = V_GLOBAL, E_GLOBAL
    ins = {
        "node_values": rng.normal(size=(V, DIM)).astype(np.float32),
        "edge_src": rng.integers(0, V, size=E).astype(np.int32),
        "edge_dest": rng.integers(0, V, size=E).astype(np.int32),
        "edge_cls": rng.integers(0, NCLS, size=E).astype(np.int32),
        "W_src": (rng.normal(size=(DIM, DIM)) / np.sqrt(DIM)).astype(np.float32),
        "b_src": np.zeros(DIM, dtype=np.float32),
        "W_dest": (rng.normal(size=(DIM, DIM)) / np.sqrt(DIM)).astype(np.float32),
        "b_dest": np.zeros(DIM, dtype=np.float32),
        "edge_emb": rng.normal(size=(NCLS, DIM)).astype(np.float32),
    }
    out = kernel(**ins)
    print("out", out.shape, out.dtype, float(np.abs(out).sum()))
